# revision 1
# baseline (speedup 1.0000x reference)
"""Trainium2 Bass kernel for nn_CrossAttention_57698590654516.

Cross-attention: B=4, N=4096 (kv len), L=1024 (q len), C=1024, H=16 heads,
D=64. Sharding: 8 cores = (batch b = core//2) x (half the heads, core%2).
Each core computes, for its batch and its 8 heads:
  kT = Wk_part @ x.T          [512, N]   (T layout, head-major rows)
  v  = x @ Wv_part.T          [N, 512]   (+ interleaved ones col for denom)
  qT = Wq_part @ query.T      [512, L]
  scoresT_h = kT_h ops        [N, L] tilewise, exp via ScalarE (no max-sub:
                              scores are O(1) by construction)
  attn_outT_h[d, l] = sum_n v[n, d] * exp(s)/denom  (denom from ones column)
  y_partial = attn_outT.T @ WprojT_part   [L, C]
Host side: y[b] = y_partial[2b] + y_partial[2b+1] + bproj.

All matmuls run in float32r (TF32-like, ~1.5e-4 rel err per GEMM, full PE
rate at free-dim >= 256). Activations / accumulation stay fp32.

The shipped kernel (build_v3) streams over N-chunks of 512: per chunk it
projects k/v and immediately runs all heads' scores/exp/attn-V against it,
accumulating unnormalized attn-out + softmax denominators in SBUF. That
overlaps kv-proj (TensorE), exp (ScalarE) and evictions (VectorE) across
the whole run: ~367us/core measured vs ~675us for the phase-serial version.
"""
import os
import sys

import numpy as np

try:
    import concourse.bass as bass  # noqa: F401
except ImportError:  # self-contained: find the repo in known locations
    for _p in ("/opt/trn_rl_repo", "/root/.axon_site/_ro/trn_rl_repo"):
        if os.path.isdir(_p) and _p not in sys.path:
            sys.path.insert(0, _p)
    import concourse.bass as bass  # noqa: F401

from contextlib import ExitStack

import concourse.tile as tile
from concourse import bacc, mybir
from concourse.bass_utils import run_bass_kernel_spmd

B, N, L, C, H = 4, 4096, 1024, 1024, 16
D = C // H  # 64
SCALE = 1.0 / float(np.sqrt(D))
P = 128
HPC = H // 2          # 8 heads per core
W = HPC * D           # 512 rows of k/v/q handled per core
F32 = mybir.dt.float32
F32R = mybir.dt.float32r
EXP = mybir.ActivationFunctionType.Exp

BF16 = mybir.dt.bfloat16


def build(reps=1, att_bf16=False, nch=256):
    NCH = nch
    NCHUNKS = N // NCH
    ADT = BF16 if att_bf16 else F32R
    nc = bacc.Bacc("TRN2", target_bir_lowering=False, debug=False, num_devices=8)
    # activations & weights arrive pre-transposed; declared float32r so they
    # feed matmuls directly (hardware uses the truncated mantissa).
    xT = nc.dram_tensor("xT", [C, N], F32R, kind="ExternalInput").ap()
    queryT = nc.dram_tensor("queryT", [C, L], F32R, kind="ExternalInput").ap()
    wkT = nc.dram_tensor("wkT", [C, W], F32R, kind="ExternalInput").ap()
    wvT = nc.dram_tensor("wvT", [C, W], F32R, kind="ExternalInput").ap()
    wqT = nc.dram_tensor("wqT", [C, W], F32R, kind="ExternalInput").ap()
    wprojT = nc.dram_tensor("wprojT", [W, C], F32R, kind="ExternalInput").ap()
    y = nc.dram_tensor("y", [L, C], F32, kind="ExternalOutput").ap()

    xT3 = xT.rearrange("(ko ki) n -> ki ko n", ki=P)          # [128, 8, N]
    queryT3 = queryT.rearrange("(ko ki) l -> ki ko l", ki=P)  # [128, 8, L]
    wkT3 = wkT.rearrange("(ko ki) m -> ki ko m", ki=P)        # [128, 8, 512]
    wvT3 = wvT.rearrange("(ko ki) m -> ki ko m", ki=P)
    wqT3 = wqT.rearrange("(ko ki) m -> ki ko m", ki=P)
    wprojT3 = wprojT.rearrange("(ko ki) c -> ki ko c", ki=P)  # [128, 4, 1024]

    with tile.TileContext(nc) as tc, ExitStack() as ctx:
      for rep in range(reps):
        persist_cm = tc.tile_pool(name=f"persist{rep}", bufs=1)
        persist = persist_cm.__enter__()
        kT = persist.tile([P, 4, N], ADT, tag="kT")
        v520 = persist.tile([P, N // P, HPC, D + 1], ADT, tag="v520")
        onesrc = persist.tile([P, 1], F32, tag="onesrc")
        nc.any.memset(onesrc[:], 1.0)
        nc.vector.tensor_copy(v520[:, :, :, D],
                              onesrc[:, 0:1].to_broadcast([P, N // P, HPC]))

        # ---------------- KV: kT = Wk @ x.T ; v = x @ Wv.T ----------------
        if att_bf16:
            with tc.tile_pool(name=f"kvw{rep}", bufs=1) as wp, \
                 tc.tile_pool(name=f"kvstg{rep}", bufs=2) as stg, \
                 tc.tile_pool(name=f"kvps{rep}", bufs=3, space="PSUM") as pp, \
                 tc.tile_pool(name=f"kvps2{rep}", bufs=2, space="PSUM") as pp2:
                wkr = wp.tile([P, 8, W], F32R, tag="wk")
                nc.sync.dma_start(wkr[:], wkT3[:])
                wvr = wp.tile([P, 8, W], F32R, tag="wv")
                nc.sync.dma_start(wvr[:], wvT3[:])
                for c in range(NCHUNKS):
                    xr = stg.tile([P, 8, NCH], F32R, tag="xs")
                    nc.sync.dma_start(xr[:], xT3[:, :, c * NCH:(c + 1) * NCH])
                    for m in range(4):
                        ps = pp.tile([P, NCH], F32, tag="kps")
                        for k in range(8):
                            nc.tensor.matmul(ps[:], wkr[:, k, m * P:(m + 1) * P],
                                             xr[:, k, :], start=(k == 0), stop=(k == 7))
                        nc.vector.tensor_copy(kT[:, m, c * NCH:(c + 1) * NCH], ps[:])
                    for t in range(NCH // P):
                        ps = pp2.tile([P, W], F32, tag="vps")
                        for k in range(8):
                            nc.tensor.matmul(ps[:], xr[:, k, t * P:(t + 1) * P],
                                             wvr[:, k, :], start=(k == 0), stop=(k == 7))
                        nt = c * (NCH // P) + t
                        nc.vector.tensor_copy(
                            v520[:, nt, :, 0:D],
                            ps[:].rearrange("p (h d) -> p h d", h=HPC))
        else:
            with tc.tile_pool(name=f"kvw{rep}", bufs=1) as wp, \
                 tc.tile_pool(name=f"kvstg{rep}", bufs=2) as stg, \
                 tc.tile_pool(name=f"kvps{rep}", bufs=3, space="PSUM") as pp:
                wkr = wp.tile([P, 8, W], F32R, tag="wk")
                nc.sync.dma_start(wkr[:], wkT3[:])
                for c in range(NCHUNKS):
                    xr = stg.tile([P, 8, NCH], F32R, tag="xs")
                    nc.sync.dma_start(xr[:], xT3[:, :, c * NCH:(c + 1) * NCH])
                    for m in range(4):
                        ps = pp.tile([P, NCH], F32, tag="kps")
                        for k in range(8):
                            nc.tensor.matmul(ps[:], wkr[:, k, m * P:(m + 1) * P],
                                             xr[:, k, :], start=(k == 0), stop=(k == 7))
                        nc.vector.tensor_copy(kT[:, m, c * NCH:(c + 1) * NCH], ps[:])

            # ---------------- KV pass 2: v = x @ Wv_part.T ----------------
            with tc.tile_pool(name=f"kvw2{rep}", bufs=1) as wp, \
                 tc.tile_pool(name=f"kvstg2{rep}", bufs=2) as stg, \
                 tc.tile_pool(name=f"kvps2{rep}", bufs=3, space="PSUM") as pp:
                wvr = wp.tile([P, 8, W], F32R, tag="wv")
                nc.sync.dma_start(wvr[:], wvT3[:])
                for c in range(NCHUNKS):
                    xr = stg.tile([P, 8, NCH], F32R, tag="xs2")
                    nc.sync.dma_start(xr[:], xT3[:, :, c * NCH:(c + 1) * NCH])
                    for t in range(NCH // P):
                        ps = pp.tile([P, W], F32, tag="vps")
                        for k in range(8):
                            nc.tensor.matmul(ps[:], xr[:, k, t * P:(t + 1) * P],
                                             wvr[:, k, :], start=(k == 0), stop=(k == 7))
                        nt = c * (NCH // P) + t
                        nc.vector.tensor_copy(
                            v520[:, nt, :, 0:D],
                            ps[:].rearrange("p (h d) -> p h d", h=HPC))

        # ---------------- Q: qT = Wq_part @ query.T ----------------
        qT = persist.tile([P, 4, L], ADT, tag="qT")
        with tc.tile_pool(name=f"qw{rep}", bufs=1) as wp, \
             tc.tile_pool(name=f"qstg{rep}", bufs=2) as stg, \
             tc.tile_pool(name=f"qps{rep}", bufs=3, space="PSUM") as pp:
            wqr = wp.tile([P, 8, W], F32R, tag="wq")
            nc.sync.dma_start(wqr[:], wqT3[:])
            for lc in range(4):
                qr = stg.tile([P, 8, 256], F32R, tag="qs")
                nc.sync.dma_start(qr[:], queryT3[:, :, lc * 256:(lc + 1) * 256])
                for m in range(4):
                    ps = pp.tile([P, 256], F32, tag="qpsum")
                    for k in range(8):
                        nc.tensor.matmul(ps[:], wqr[:, k, m * P:(m + 1) * P],
                                         qr[:, k, :], start=(k == 0), stop=(k == 7))
                    nc.vector.tensor_copy(qT[:, m, lc * 256:(lc + 1) * 256], ps[:])

        # ---------------- Attention ----------------
        aoT = persist.tile([P, 4, L], F32R, tag="aoT")
        with tc.tile_pool(name=f"probs{rep}", bufs=3) as probs_pool, \
             tc.tile_pool(name=f"attsm{rep}", bufs=2) as small, \
             tc.tile_pool(name=f"spsum{rep}", bufs=3, space="PSUM") as spsum, \
             tc.tile_pool(name=f"apsum{rep}", bufs=2, space="PSUM") as apsum:
            for h in range(HPC):
                bp = D * (h % 2)      # 0 or 64: partition base within pair
                pr = h // 2           # pair index
                apts = [apsum.tile([D + 1, 512], F32, tag="apt",
                                   name=f"apt_{rep}_{h}_{i}") for i in range(2)]
                for n in range(N // P):
                    spt = spsum.tile([P, 1024], F32, tag="spt")
                    for lc in range(2):
                        nc.tensor.matmul(
                            spt[:, lc * 512:(lc + 1) * 512],
                            kT[bp:bp + D, pr, n * P:(n + 1) * P],
                            qT[bp:bp + D, pr, lc * 512:(lc + 1) * 512],
                            start=True, stop=True)
                    pt = probs_pool.tile([P, 1024], ADT, tag="pt")
                    nc.scalar.activation(pt[:], spt[:], EXP, scale=SCALE)
                    for lc in range(2):
                        nc.tensor.matmul(
                            apts[lc][:], v520[:, n, h, :],
                            pt[:, lc * 512:(lc + 1) * 512],
                            start=(n == 0), stop=(n == N // P - 1))
                for lc in range(2):
                    apt = apts[lc]
                    r64t = small.tile([P, 512], F32, tag="r64")
                    nc.vector.reciprocal(r64t[D:D + 1, :], apt[D:D + 1, :])
                    rrow = small.tile([1, 512], F32, tag="rrow")
                    nc.sync.dma_start(rrow[:], r64t[D:D + 1, :])
                    rb = small.tile([D, 512], F32, tag="rb")
                    nc.gpsimd.partition_broadcast(rb[:], rrow[:])
                    dst = aoT[bp:bp + D, pr, lc * 512:(lc + 1) * 512]
                    if bp == 0:
                        nc.vector.tensor_mul(dst, apt[0:D, :], rb[:])
                    else:
                        tmp = small.tile([D, 512], F32R, tag="aotmp")
                        nc.vector.tensor_mul(tmp[:], apt[0:D, :], rb[:])
                        nc.sync.dma_start(dst, tmp[:])

        # -------- Proj: y_partial = attn_outT.T @ WprojT --------
        with tc.tile_pool(name=f"pw{rep}", bufs=1) as wp, \
             tc.tile_pool(name=f"ypool{rep}", bufs=3) as ypool, \
             tc.tile_pool(name=f"pps{rep}", bufs=3, space="PSUM") as pp:
            wpr = wp.tile([P, 4, C], F32R, tag="wp")
            nc.sync.dma_start(wpr[:], wprojT3[:])
            for l in range(L // P):
                for co in range(2):
                    ps = pp.tile([P, 512], F32, tag="yps")
                    for ci in range(4):
                        nc.tensor.matmul(ps[:], aoT[:, ci, l * P:(l + 1) * P],
                                         wpr[:, ci, co * 512:(co + 1) * 512],
                                         start=(ci == 0), stop=(ci == 3))
                    yt = ypool.tile([P, 512], F32, tag="yt")
                    nc.vector.tensor_copy(yt[:], ps[:])
                    nc.sync.dma_start(y[l * P:(l + 1) * P, co * 512:(co + 1) * 512],
                                      yt[:])
        persist_cm.__exit__(None, None, None)
    nc.finalize()
    return nc



def build_v3(reps=1, nch=512):
    """Streaming kernel: q proj first, then one loop over N-chunks that
    projects k/v for the chunk and immediately runs all heads' attention
    against it, accumulating unnormalized attn-out (+denom) in SBUF.
    Overlaps kv-proj (PE), exp (ACT) and evictions (DVE) across the whole
    run instead of serializing phases."""
    NCH = nch
    NT_PER = NCH // P
    NCHUNKS = N // NCH
    nc = bacc.Bacc("TRN2", target_bir_lowering=False, debug=False, num_devices=8)
    xT = nc.dram_tensor("xT", [C, N], F32R, kind="ExternalInput").ap()
    queryT = nc.dram_tensor("queryT", [C, L], F32R, kind="ExternalInput").ap()
    wkT = nc.dram_tensor("wkT", [C, W], F32R, kind="ExternalInput").ap()
    wvT = nc.dram_tensor("wvT", [C, W], F32R, kind="ExternalInput").ap()
    wqT = nc.dram_tensor("wqT", [C, W], F32R, kind="ExternalInput").ap()
    wprojT = nc.dram_tensor("wprojT", [W, C], F32R, kind="ExternalInput").ap()
    y = nc.dram_tensor("y", [L, C], F32, kind="ExternalOutput").ap()

    xT3 = xT.rearrange("(ko ki) n -> ki ko n", ki=P)
    queryT3 = queryT.rearrange("(ko ki) l -> ki ko l", ki=P)
    wkT3 = wkT.rearrange("(ko ki) m -> ki ko m", ki=P)
    wvT3 = wvT.rearrange("(ko ki) m -> ki ko m", ki=P)
    wqT3 = wqT.rearrange("(ko ki) m -> ki ko m", ki=P)
    wprojT3 = wprojT.rearrange("(ko ki) c -> ki ko c", ki=P)

    with tile.TileContext(nc) as tc, ExitStack() as ctx:
      for rep in range(reps):
        persist_cm = tc.tile_pool(name=f"v3p{rep}", bufs=1)
        persist = persist_cm.__enter__()
        qT = persist.tile([P, 4, L], F32R, tag="qT")
        aoT = persist.tile([P, 4, L], F32R, tag="aoT")
        accumA = persist.tile([D + 1, 2 * HPC, 512], F32, tag="accumA")
        onesrc = persist.tile([P, 1], F32, tag="onesrc")
        nc.any.memset(onesrc[:], 1.0)
        nc.any.memset(accumA[:], 0.0)

        # ---- Q: qT = Wq_part @ query.T ----
        with tc.tile_pool(name=f"v3qw{rep}", bufs=1) as wp, \
             tc.tile_pool(name=f"v3qs{rep}", bufs=2) as stg, \
             tc.tile_pool(name=f"v3qp{rep}", bufs=3, space="PSUM") as pp:
            wqr = wp.tile([P, 8, W], F32R, tag="wq")
            nc.sync.dma_start(wqr[:], wqT3[:])
            for lc in range(2):
                qr = stg.tile([P, 8, 512], F32R, tag="qs")
                nc.sync.dma_start(qr[:], queryT3[:, :, lc * 512:(lc + 1) * 512])
                for m in range(4):
                    ps = pp.tile([P, 512], F32, tag="qps")
                    for k in range(8):
                        nc.tensor.matmul(ps[:], wqr[:, k, m * P:(m + 1) * P],
                                         qr[:, k, :], start=(k == 0), stop=(k == 7))
                    nc.vector.tensor_copy(qT[:, m, lc * 512:(lc + 1) * 512], ps[:])

        # ---- streaming kv-proj + attention ----
        with tc.tile_pool(name=f"v3w{rep}", bufs=1) as wp, \
             tc.tile_pool(name=f"v3x{rep}", bufs=2) as xstg, \
             tc.tile_pool(name=f"v3kv{rep}", bufs=2) as kvp, \
             tc.tile_pool(name=f"v3pr{rep}", bufs=3) as probs_pool, \
             tc.tile_pool(name=f"v3sm{rep}", bufs=2) as small, \
             tc.tile_pool(name=f"v3kvps{rep}", bufs=2, space="PSUM") as kvps, \
             tc.tile_pool(name=f"v3sps{rep}", bufs=2, space="PSUM") as spsum, \
             tc.tile_pool(name=f"v3aps{rep}", bufs=2, space="PSUM") as apsum:
            wkr = wp.tile([P, 8, W], F32R, tag="wk")
            nc.sync.dma_start(wkr[:], wkT3[:])
            wvr = wp.tile([P, 8, W], F32R, tag="wv")
            nc.sync.dma_start(wvr[:], wvT3[:])
            for c in range(NCHUNKS):
                xr = xstg.tile([P, 8, NCH], F32R, tag="xs")
                nc.sync.dma_start(xr[:], xT3[:, :, c * NCH:(c + 1) * NCH])
                kTc = kvp.tile([P, 4, NCH], F32R, tag="kTc")
                for m in range(4):
                    ps = kvps.tile([P, W], F32, tag="kvpsum",
                                   name=f"kps_{rep}_{c}_{m}")
                    for k in range(8):
                        nc.tensor.matmul(ps[:, :NCH], wkr[:, k, m * P:(m + 1) * P],
                                         xr[:, k, :], start=(k == 0), stop=(k == 7))
                    nc.vector.tensor_copy(kTc[:, m, :], ps[:, :NCH])
                v520c = kvp.tile([P, NT_PER, HPC, D + 1], F32R, tag="v520c")
                nc.vector.tensor_copy(
                    v520c[:, :, :, D],
                    onesrc[:, 0:1].to_broadcast([P, NT_PER, HPC]))
                for t in range(NT_PER):
                    ps = kvps.tile([P, W], F32, tag="kvpsum",
                                   name=f"vps_{rep}_{c}_{t}")
                    for k in range(8):
                        nc.tensor.matmul(ps[:], xr[:, k, t * P:(t + 1) * P],
                                         wvr[:, k, :], start=(k == 0), stop=(k == 7))
                    nc.vector.tensor_copy(
                        v520c[:, t, :, 0:D],
                        ps[:].rearrange("p (h d) -> p h d", h=HPC))
                for h in range(HPC):
                    bp = D * (h % 2)
                    pr = h // 2
                    apts = [apsum.tile([D + 1, 512], F32, tag="apt",
                                       name=f"apt_{rep}_{c}_{h}_{i}")
                            for i in range(2)]
                    for t in range(NT_PER):
                        spt = spsum.tile([P, 1024], F32, tag="spt")
                        for lc in range(2):
                            nc.tensor.matmul(
                                spt[:, lc * 512:(lc + 1) * 512],
                                kTc[bp:bp + D, pr, t * P:(t + 1) * P],
                                qT[bp:bp + D, pr, lc * 512:(lc + 1) * 512],
                                start=True, stop=True)
                        pt = probs_pool.tile([P, 1024], F32R, tag="pt")
                        nc.scalar.activation(pt[:], spt[:], EXP, scale=SCALE)
                        for lc in range(2):
                            nc.tensor.matmul(
                                apts[lc][:], v520c[:, t, h, :],
                                pt[:, lc * 512:(lc + 1) * 512],
                                start=(t == 0), stop=(t == NT_PER - 1))
                    for lc in range(2):
                        idx = h * 2 + lc
                        nc.vector.tensor_add(accumA[:, idx, :],
                                             accumA[:, idx, :], apts[lc][:])
            # normalize + write aoT
            for h in range(HPC):
                bp = D * (h % 2)
                pr = h // 2
                for lc in range(2):
                    idx = h * 2 + lc
                    r64t = small.tile([P, 512], F32, tag="r64")
                    nc.vector.reciprocal(r64t[D:D + 1, :], accumA[D:D + 1, idx, :])
                    rrow = small.tile([1, 512], F32, tag="rrow")
                    nc.sync.dma_start(rrow[:], r64t[D:D + 1, :])
                    rb = small.tile([D, 512], F32, tag="rb")
                    nc.gpsimd.partition_broadcast(rb[:], rrow[:])
                    dst = aoT[bp:bp + D, pr, lc * 512:(lc + 1) * 512]
                    if bp == 0:
                        nc.vector.tensor_mul(dst, accumA[0:D, idx, :], rb[:])
                    else:
                        tmp = small.tile([D, 512], F32R, tag="aotmp")
                        nc.vector.tensor_mul(tmp[:], accumA[0:D, idx, :], rb[:])
                        nc.sync.dma_start(dst, tmp[:])

        # ---- proj ----
        with tc.tile_pool(name=f"v3pw{rep}", bufs=1) as wp, \
             tc.tile_pool(name=f"v3y{rep}", bufs=3) as ypool, \
             tc.tile_pool(name=f"v3pp{rep}", bufs=3, space="PSUM") as pp:
            wpr = wp.tile([P, 4, C], F32R, tag="wp")
            nc.sync.dma_start(wpr[:], wprojT3[:])
            for l in range(L // P):
                for co in range(2):
                    ps = pp.tile([P, 512], F32, tag="yps")
                    for ci in range(4):
                        nc.tensor.matmul(ps[:], aoT[:, ci, l * P:(l + 1) * P],
                                         wpr[:, ci, co * 512:(co + 1) * 512],
                                         start=(ci == 0), stop=(ci == 3))
                    yt = ypool.tile([P, 512], F32, tag="yt")
                    nc.vector.tensor_copy(yt[:], ps[:])
                    nc.sync.dma_start(y[l * P:(l + 1) * P, co * 512:(co + 1) * 512],
                                      yt[:])
        persist_cm.__exit__(None, None, None)
    nc.finalize()
    return nc

_NC_CACHE = {}


def get_nc():
    if "nc" not in _NC_CACHE:
        _NC_CACHE["nc"] = build_v3()
    return _NC_CACHE["nc"]


def make_in_maps(x, query, Wq, Wkv, Wproj):
    x = np.asarray(x, dtype=np.float32)
    query = np.asarray(query, dtype=np.float32)
    Wq = np.asarray(Wq, dtype=np.float32)
    Wkv = np.asarray(Wkv, dtype=np.float32)
    Wproj = np.asarray(Wproj, dtype=np.float32)
    in_maps = []
    for core in range(8):
        b, half = core // 2, core % 2
        h0 = half * W  # 0 or 512: channel offset of this core's heads
        in_maps.append({
            "xT": np.ascontiguousarray(x[b].T),
            "queryT": np.ascontiguousarray(query[b].T),
            "wkT": np.ascontiguousarray(Wkv[h0:h0 + W, :].T),
            "wvT": np.ascontiguousarray(Wkv[C + h0:C + h0 + W, :].T),
            "wqT": np.ascontiguousarray(Wq[h0:h0 + W, :].T),
            "wprojT": np.ascontiguousarray(Wproj[:, h0:h0 + W].T),
        })
    return in_maps


def combine(results, bproj):
    y = np.zeros((B, L, C), np.float32)
    for core in range(8):
        y[core // 2] += results[core]["y"]
    y += np.asarray(bproj, dtype=np.float32)[None, None, :]
    return y


def kernel(x, query, Wq, Wkv, Wproj, bproj):
    nc = get_nc()
    in_maps = make_in_maps(x, query, Wq, Wkv, Wproj)
    res = run_bass_kernel_spmd(nc, in_maps, core_ids=list(range(8)))
    return combine(res.results, bproj)



# revision 3
# speedup vs baseline: 1.4298x; 1.4298x over previous
"""Trainium2 Bass kernel for nn_CrossAttention_57698590654516.

Cross-attention: B=4, N=4096 (kv len), L=1024 (q len), C=1024, H=16 heads,
D=64. Sharding: 8 cores = (batch b = core//2) x (half the heads, core%2).
Each core computes, for its batch and its 8 heads:
  kT = Wk_part @ x.T          [512, N]   (T layout, head-major rows)
  v  = x @ Wv_part.T          [N, 512]   (+ interleaved ones col for denom)
  qT = Wq_part @ query.T      [512, L]
  scoresT_h = kT_h ops        [N, L] tilewise, exp via ScalarE (no max-sub:
                              scores are O(1) by construction)
  attn_outT_h[d, l] = sum_n v[n, d] * exp(s)/denom  (denom from ones column)
  y_partial = attn_outT.T @ WprojT_part   [L, C]
Host side: y[b] = y_partial[2b] + y_partial[2b+1] + bproj.

All matmuls run in float32r (TF32-like, ~1.5e-4 rel err per GEMM, full PE
rate at free-dim >= 256). Activations / accumulation stay fp32.

The shipped kernel (build_v3) streams over N-chunks of 512: per chunk it
projects k/v and immediately runs all heads' scores/exp/attn-V against it,
accumulating unnormalized attn-out + softmax denominators in SBUF. That
overlaps kv-proj (TensorE), exp (ScalarE) and evictions (VectorE) across
the whole run: ~367us/core measured vs ~675us for the phase-serial version.
"""
import os
import sys

import numpy as np

try:
    import concourse.bass as bass  # noqa: F401
except ImportError:  # self-contained: find the repo in known locations
    for _p in ("/opt/trn_rl_repo", "/root/.axon_site/_ro/trn_rl_repo"):
        if os.path.isdir(_p) and _p not in sys.path:
            sys.path.insert(0, _p)
    import concourse.bass as bass  # noqa: F401

from contextlib import ExitStack

import concourse.tile as tile
from concourse import bacc, mybir
from concourse.bass_utils import run_bass_kernel_spmd

B, N, L, C, H = 4, 4096, 1024, 1024, 16
D = C // H  # 64
SCALE = 1.0 / float(np.sqrt(D))
P = 128
HPC = H // 2          # 8 heads per core
W = HPC * D           # 512 rows of k/v/q handled per core
F32 = mybir.dt.float32
F32R = mybir.dt.float32r
EXP = mybir.ActivationFunctionType.Exp

BF16 = mybir.dt.bfloat16


def build(reps=1, att_bf16=False, nch=256):
    NCH = nch
    NCHUNKS = N // NCH
    ADT = BF16 if att_bf16 else F32R
    nc = bacc.Bacc("TRN2", target_bir_lowering=False, debug=False, num_devices=8)
    # activations & weights arrive pre-transposed; declared float32r so they
    # feed matmuls directly (hardware uses the truncated mantissa).
    xT = nc.dram_tensor("xT", [C, N], F32R, kind="ExternalInput").ap()
    queryT = nc.dram_tensor("queryT", [C, L], F32R, kind="ExternalInput").ap()
    wkT = nc.dram_tensor("wkT", [C, W], F32R, kind="ExternalInput").ap()
    wvT = nc.dram_tensor("wvT", [C, W], F32R, kind="ExternalInput").ap()
    wqT = nc.dram_tensor("wqT", [C, W], F32R, kind="ExternalInput").ap()
    wprojT = nc.dram_tensor("wprojT", [W, C], F32R, kind="ExternalInput").ap()
    y = nc.dram_tensor("y", [L, C], F32, kind="ExternalOutput").ap()

    xT3 = xT.rearrange("(ko ki) n -> ki ko n", ki=P)          # [128, 8, N]
    queryT3 = queryT.rearrange("(ko ki) l -> ki ko l", ki=P)  # [128, 8, L]
    wkT3 = wkT.rearrange("(ko ki) m -> ki ko m", ki=P)        # [128, 8, 512]
    wvT3 = wvT.rearrange("(ko ki) m -> ki ko m", ki=P)
    wqT3 = wqT.rearrange("(ko ki) m -> ki ko m", ki=P)
    wprojT3 = wprojT.rearrange("(ko ki) c -> ki ko c", ki=P)  # [128, 4, 1024]

    with tile.TileContext(nc) as tc, ExitStack() as ctx:
      for rep in range(reps):
        persist_cm = tc.tile_pool(name=f"persist{rep}", bufs=1)
        persist = persist_cm.__enter__()
        kT = persist.tile([P, 4, N], ADT, tag="kT")
        v520 = persist.tile([P, N // P, HPC, D + 1], ADT, tag="v520")
        onesrc = persist.tile([P, 1], F32, tag="onesrc")
        nc.any.memset(onesrc[:], 1.0)
        nc.vector.tensor_copy(v520[:, :, :, D],
                              onesrc[:, 0:1].to_broadcast([P, N // P, HPC]))

        # ---------------- KV: kT = Wk @ x.T ; v = x @ Wv.T ----------------
        if att_bf16:
            with tc.tile_pool(name=f"kvw{rep}", bufs=1) as wp, \
                 tc.tile_pool(name=f"kvstg{rep}", bufs=2) as stg, \
                 tc.tile_pool(name=f"kvps{rep}", bufs=3, space="PSUM") as pp, \
                 tc.tile_pool(name=f"kvps2{rep}", bufs=2, space="PSUM") as pp2:
                wkr = wp.tile([P, 8, W], F32R, tag="wk")
                nc.sync.dma_start(wkr[:], wkT3[:])
                wvr = wp.tile([P, 8, W], F32R, tag="wv")
                nc.sync.dma_start(wvr[:], wvT3[:])
                for c in range(NCHUNKS):
                    xr = stg.tile([P, 8, NCH], F32R, tag="xs")
                    nc.sync.dma_start(xr[:], xT3[:, :, c * NCH:(c + 1) * NCH])
                    for m in range(4):
                        ps = pp.tile([P, NCH], F32, tag="kps")
                        for k in range(8):
                            nc.tensor.matmul(ps[:], wkr[:, k, m * P:(m + 1) * P],
                                             xr[:, k, :], start=(k == 0), stop=(k == 7))
                        nc.vector.tensor_copy(kT[:, m, c * NCH:(c + 1) * NCH], ps[:])
                    for t in range(NCH // P):
                        ps = pp2.tile([P, W], F32, tag="vps")
                        for k in range(8):
                            nc.tensor.matmul(ps[:], xr[:, k, t * P:(t + 1) * P],
                                             wvr[:, k, :], start=(k == 0), stop=(k == 7))
                        nt = c * (NCH // P) + t
                        nc.vector.tensor_copy(
                            v520[:, nt, :, 0:D],
                            ps[:].rearrange("p (h d) -> p h d", h=HPC))
        else:
            with tc.tile_pool(name=f"kvw{rep}", bufs=1) as wp, \
                 tc.tile_pool(name=f"kvstg{rep}", bufs=2) as stg, \
                 tc.tile_pool(name=f"kvps{rep}", bufs=3, space="PSUM") as pp:
                wkr = wp.tile([P, 8, W], F32R, tag="wk")
                nc.sync.dma_start(wkr[:], wkT3[:])
                for c in range(NCHUNKS):
                    xr = stg.tile([P, 8, NCH], F32R, tag="xs")
                    nc.sync.dma_start(xr[:], xT3[:, :, c * NCH:(c + 1) * NCH])
                    for m in range(4):
                        ps = pp.tile([P, NCH], F32, tag="kps")
                        for k in range(8):
                            nc.tensor.matmul(ps[:], wkr[:, k, m * P:(m + 1) * P],
                                             xr[:, k, :], start=(k == 0), stop=(k == 7))
                        nc.vector.tensor_copy(kT[:, m, c * NCH:(c + 1) * NCH], ps[:])

            # ---------------- KV pass 2: v = x @ Wv_part.T ----------------
            with tc.tile_pool(name=f"kvw2{rep}", bufs=1) as wp, \
                 tc.tile_pool(name=f"kvstg2{rep}", bufs=2) as stg, \
                 tc.tile_pool(name=f"kvps2{rep}", bufs=3, space="PSUM") as pp:
                wvr = wp.tile([P, 8, W], F32R, tag="wv")
                nc.sync.dma_start(wvr[:], wvT3[:])
                for c in range(NCHUNKS):
                    xr = stg.tile([P, 8, NCH], F32R, tag="xs2")
                    nc.sync.dma_start(xr[:], xT3[:, :, c * NCH:(c + 1) * NCH])
                    for t in range(NCH // P):
                        ps = pp.tile([P, W], F32, tag="vps")
                        for k in range(8):
                            nc.tensor.matmul(ps[:], xr[:, k, t * P:(t + 1) * P],
                                             wvr[:, k, :], start=(k == 0), stop=(k == 7))
                        nt = c * (NCH // P) + t
                        nc.vector.tensor_copy(
                            v520[:, nt, :, 0:D],
                            ps[:].rearrange("p (h d) -> p h d", h=HPC))

        # ---------------- Q: qT = Wq_part @ query.T ----------------
        qT = persist.tile([P, 4, L], ADT, tag="qT")
        with tc.tile_pool(name=f"qw{rep}", bufs=1) as wp, \
             tc.tile_pool(name=f"qstg{rep}", bufs=2) as stg, \
             tc.tile_pool(name=f"qps{rep}", bufs=3, space="PSUM") as pp:
            wqr = wp.tile([P, 8, W], F32R, tag="wq")
            nc.sync.dma_start(wqr[:], wqT3[:])
            for lc in range(4):
                qr = stg.tile([P, 8, 256], F32R, tag="qs")
                nc.sync.dma_start(qr[:], queryT3[:, :, lc * 256:(lc + 1) * 256])
                for m in range(4):
                    ps = pp.tile([P, 256], F32, tag="qpsum")
                    for k in range(8):
                        nc.tensor.matmul(ps[:], wqr[:, k, m * P:(m + 1) * P],
                                         qr[:, k, :], start=(k == 0), stop=(k == 7))
                    nc.vector.tensor_copy(qT[:, m, lc * 256:(lc + 1) * 256], ps[:])

        # ---------------- Attention ----------------
        aoT = persist.tile([P, 4, L], F32R, tag="aoT")
        with tc.tile_pool(name=f"probs{rep}", bufs=3) as probs_pool, \
             tc.tile_pool(name=f"attsm{rep}", bufs=2) as small, \
             tc.tile_pool(name=f"spsum{rep}", bufs=3, space="PSUM") as spsum, \
             tc.tile_pool(name=f"apsum{rep}", bufs=2, space="PSUM") as apsum:
            for h in range(HPC):
                bp = D * (h % 2)      # 0 or 64: partition base within pair
                pr = h // 2           # pair index
                apts = [apsum.tile([D + 1, 512], F32, tag="apt",
                                   name=f"apt_{rep}_{h}_{i}") for i in range(2)]
                for n in range(N // P):
                    spt = spsum.tile([P, 1024], F32, tag="spt")
                    for lc in range(2):
                        nc.tensor.matmul(
                            spt[:, lc * 512:(lc + 1) * 512],
                            kT[bp:bp + D, pr, n * P:(n + 1) * P],
                            qT[bp:bp + D, pr, lc * 512:(lc + 1) * 512],
                            start=True, stop=True)
                    pt = probs_pool.tile([P, 1024], ADT, tag="pt")
                    nc.scalar.activation(pt[:], spt[:], EXP, scale=SCALE)
                    for lc in range(2):
                        nc.tensor.matmul(
                            apts[lc][:], v520[:, n, h, :],
                            pt[:, lc * 512:(lc + 1) * 512],
                            start=(n == 0), stop=(n == N // P - 1))
                for lc in range(2):
                    apt = apts[lc]
                    r64t = small.tile([P, 512], F32, tag="r64")
                    nc.vector.reciprocal(r64t[D:D + 1, :], apt[D:D + 1, :])
                    rrow = small.tile([1, 512], F32, tag="rrow")
                    nc.sync.dma_start(rrow[:], r64t[D:D + 1, :])
                    rb = small.tile([D, 512], F32, tag="rb")
                    nc.gpsimd.partition_broadcast(rb[:], rrow[:])
                    dst = aoT[bp:bp + D, pr, lc * 512:(lc + 1) * 512]
                    if bp == 0:
                        nc.vector.tensor_mul(dst, apt[0:D, :], rb[:])
                    else:
                        tmp = small.tile([D, 512], F32R, tag="aotmp")
                        nc.vector.tensor_mul(tmp[:], apt[0:D, :], rb[:])
                        nc.sync.dma_start(dst, tmp[:])

        # -------- Proj: y_partial = attn_outT.T @ WprojT --------
        with tc.tile_pool(name=f"pw{rep}", bufs=1) as wp, \
             tc.tile_pool(name=f"ypool{rep}", bufs=3) as ypool, \
             tc.tile_pool(name=f"pps{rep}", bufs=3, space="PSUM") as pp:
            wpr = wp.tile([P, 4, C], F32R, tag="wp")
            nc.sync.dma_start(wpr[:], wprojT3[:])
            for l in range(L // P):
                for co in range(2):
                    ps = pp.tile([P, 512], F32, tag="yps")
                    for ci in range(4):
                        nc.tensor.matmul(ps[:], aoT[:, ci, l * P:(l + 1) * P],
                                         wpr[:, ci, co * 512:(co + 1) * 512],
                                         start=(ci == 0), stop=(ci == 3))
                    yt = ypool.tile([P, 512], F32, tag="yt")
                    nc.vector.tensor_copy(yt[:], ps[:])
                    nc.sync.dma_start(y[l * P:(l + 1) * P, co * 512:(co + 1) * 512],
                                      yt[:])
        persist_cm.__exit__(None, None, None)
    nc.finalize()
    return nc



def build_v3(reps=1, nch=512):
    """Streaming kernel: q proj first, then one loop over N-chunks that
    projects k/v for the chunk and immediately runs all heads' attention
    against it, accumulating unnormalized attn-out (+denom) in SBUF.
    Overlaps kv-proj (PE), exp (ACT) and evictions (DVE) across the whole
    run instead of serializing phases."""
    NCH = nch
    NT_PER = NCH // P
    NCHUNKS = N // NCH
    nc = bacc.Bacc("TRN2", target_bir_lowering=False, debug=False, num_devices=8)
    xT = nc.dram_tensor("xT", [C, N], F32R, kind="ExternalInput").ap()
    queryT = nc.dram_tensor("queryT", [C, L], F32R, kind="ExternalInput").ap()
    wkT = nc.dram_tensor("wkT", [C, W], F32R, kind="ExternalInput").ap()
    wvT = nc.dram_tensor("wvT", [C, W], F32R, kind="ExternalInput").ap()
    wqT = nc.dram_tensor("wqT", [C, W], F32R, kind="ExternalInput").ap()
    wprojT = nc.dram_tensor("wprojT", [W, C], F32R, kind="ExternalInput").ap()
    y = nc.dram_tensor("y", [L, C], F32, kind="ExternalOutput").ap()

    xT3 = xT.rearrange("(ko ki) n -> ki ko n", ki=P)
    queryT3 = queryT.rearrange("(ko ki) l -> ki ko l", ki=P)
    wkT3 = wkT.rearrange("(ko ki) m -> ki ko m", ki=P)
    wvT3 = wvT.rearrange("(ko ki) m -> ki ko m", ki=P)
    wqT3 = wqT.rearrange("(ko ki) m -> ki ko m", ki=P)
    wprojT3 = wprojT.rearrange("(ko ki) c -> ki ko c", ki=P)

    with tile.TileContext(nc) as tc, ExitStack() as ctx:
      for rep in range(reps):
        persist_cm = tc.tile_pool(name=f"v3p{rep}", bufs=1)
        persist = persist_cm.__enter__()
        qT = persist.tile([P, 4, L], F32R, tag="qT")
        aoT = persist.tile([P, 4, L], F32R, tag="aoT")
        accumA = persist.tile([D + 1, 2 * HPC, 512], F32, tag="accumA")
        onesrc = persist.tile([P, 1], F32, tag="onesrc")
        nc.any.memset(onesrc[:], 1.0)
        nc.any.memset(accumA[:], 0.0)

        # ---- Q: qT = Wq_part @ query.T ----
        with tc.tile_pool(name=f"v3qw{rep}", bufs=1) as wp, \
             tc.tile_pool(name=f"v3qs{rep}", bufs=2) as stg, \
             tc.tile_pool(name=f"v3qp{rep}", bufs=3, space="PSUM") as pp:
            wqr = wp.tile([P, 8, W], F32R, tag="wq")
            nc.sync.dma_start(wqr[:], wqT3[:])
            for lc in range(2):
                qr = stg.tile([P, 8, 512], F32R, tag="qs")
                nc.sync.dma_start(qr[:], queryT3[:, :, lc * 512:(lc + 1) * 512])
                for m in range(4):
                    ps = pp.tile([P, 512], F32, tag="qps")
                    for k in range(8):
                        nc.tensor.matmul(ps[:], wqr[:, k, m * P:(m + 1) * P],
                                         qr[:, k, :], start=(k == 0), stop=(k == 7))
                    nc.vector.tensor_copy(qT[:, m, lc * 512:(lc + 1) * 512], ps[:])

        # ---- streaming kv-proj + attention ----
        with tc.tile_pool(name=f"v3w{rep}", bufs=1) as wp, \
             tc.tile_pool(name=f"v3x{rep}", bufs=2) as xstg, \
             tc.tile_pool(name=f"v3kv{rep}", bufs=2) as kvp, \
             tc.tile_pool(name=f"v3pr{rep}", bufs=3) as probs_pool, \
             tc.tile_pool(name=f"v3sm{rep}", bufs=2) as small, \
             tc.tile_pool(name=f"v3kvps{rep}", bufs=2, space="PSUM") as kvps, \
             tc.tile_pool(name=f"v3sps{rep}", bufs=2, space="PSUM") as spsum, \
             tc.tile_pool(name=f"v3aps{rep}", bufs=2, space="PSUM") as apsum:
            wkr = wp.tile([P, 8, W], F32R, tag="wk")
            nc.sync.dma_start(wkr[:], wkT3[:])
            wvr = wp.tile([P, 8, W], F32R, tag="wv")
            nc.sync.dma_start(wvr[:], wvT3[:])
            for c in range(NCHUNKS):
                xr = xstg.tile([P, 8, NCH], F32R, tag="xs")
                nc.sync.dma_start(xr[:], xT3[:, :, c * NCH:(c + 1) * NCH])
                kTc = kvp.tile([P, 4, NCH], F32R, tag="kTc")
                for m in range(4):
                    ps = kvps.tile([P, W], F32, tag="kvpsum",
                                   name=f"kps_{rep}_{c}_{m}")
                    for k in range(8):
                        nc.tensor.matmul(ps[:, :NCH], wkr[:, k, m * P:(m + 1) * P],
                                         xr[:, k, :], start=(k == 0), stop=(k == 7))
                    nc.vector.tensor_copy(kTc[:, m, :], ps[:, :NCH])
                v520c = kvp.tile([P, NT_PER, HPC, D + 1], F32R, tag="v520c")
                nc.vector.tensor_copy(
                    v520c[:, :, :, D],
                    onesrc[:, 0:1].to_broadcast([P, NT_PER, HPC]))
                for t in range(NT_PER):
                    ps = kvps.tile([P, W], F32, tag="kvpsum",
                                   name=f"vps_{rep}_{c}_{t}")
                    for k in range(8):
                        nc.tensor.matmul(ps[:], xr[:, k, t * P:(t + 1) * P],
                                         wvr[:, k, :], start=(k == 0), stop=(k == 7))
                    nc.vector.tensor_copy(
                        v520c[:, t, :, 0:D],
                        ps[:].rearrange("p (h d) -> p h d", h=HPC))
                for h in range(HPC):
                    bp = D * (h % 2)
                    pr = h // 2
                    apts = [apsum.tile([D + 1, 512], F32, tag="apt",
                                       name=f"apt_{rep}_{c}_{h}_{i}")
                            for i in range(2)]
                    for t in range(NT_PER):
                        spt = spsum.tile([P, 1024], F32, tag="spt")
                        for lc in range(2):
                            nc.tensor.matmul(
                                spt[:, lc * 512:(lc + 1) * 512],
                                kTc[bp:bp + D, pr, t * P:(t + 1) * P],
                                qT[bp:bp + D, pr, lc * 512:(lc + 1) * 512],
                                start=True, stop=True)
                        pt = probs_pool.tile([P, 1024], F32R, tag="pt")
                        nc.scalar.activation(pt[:], spt[:], EXP, scale=SCALE)
                        for lc in range(2):
                            nc.tensor.matmul(
                                apts[lc][:], v520c[:, t, h, :],
                                pt[:, lc * 512:(lc + 1) * 512],
                                start=(t == 0), stop=(t == NT_PER - 1))
                    for lc in range(2):
                        idx = h * 2 + lc
                        nc.vector.tensor_add(accumA[:, idx, :],
                                             accumA[:, idx, :], apts[lc][:])
            # normalize + write aoT
            for h in range(HPC):
                bp = D * (h % 2)
                pr = h // 2
                for lc in range(2):
                    idx = h * 2 + lc
                    r64t = small.tile([P, 512], F32, tag="r64")
                    nc.vector.reciprocal(r64t[D:D + 1, :], accumA[D:D + 1, idx, :])
                    rrow = small.tile([1, 512], F32, tag="rrow")
                    nc.sync.dma_start(rrow[:], r64t[D:D + 1, :])
                    rb = small.tile([D, 512], F32, tag="rb")
                    nc.gpsimd.partition_broadcast(rb[:], rrow[:])
                    dst = aoT[bp:bp + D, pr, lc * 512:(lc + 1) * 512]
                    if bp == 0:
                        nc.vector.tensor_mul(dst, accumA[0:D, idx, :], rb[:])
                    else:
                        tmp = small.tile([D, 512], F32R, tag="aotmp")
                        nc.vector.tensor_mul(tmp[:], accumA[0:D, idx, :], rb[:])
                        nc.sync.dma_start(dst, tmp[:])

        # ---- proj ----
        with tc.tile_pool(name=f"v3pw{rep}", bufs=1) as wp, \
             tc.tile_pool(name=f"v3y{rep}", bufs=3) as ypool, \
             tc.tile_pool(name=f"v3pp{rep}", bufs=3, space="PSUM") as pp:
            wpr = wp.tile([P, 4, C], F32R, tag="wp")
            nc.sync.dma_start(wpr[:], wprojT3[:])
            for l in range(L // P):
                for co in range(2):
                    ps = pp.tile([P, 512], F32, tag="yps")
                    for ci in range(4):
                        nc.tensor.matmul(ps[:], aoT[:, ci, l * P:(l + 1) * P],
                                         wpr[:, ci, co * 512:(co + 1) * 512],
                                         start=(ci == 0), stop=(ci == 3))
                    yt = ypool.tile([P, 512], F32, tag="yt")
                    nc.vector.tensor_copy(yt[:], ps[:])
                    nc.sync.dma_start(y[l * P:(l + 1) * P, co * 512:(co + 1) * 512],
                                      yt[:])
        persist_cm.__exit__(None, None, None)
    nc.finalize()
    return nc

def build_v4(reps=1, nch=512):
    """v3 + (a) row-packed score matmuls: the two heads of a pair live on
    SBUF partitions 0-63 / 64-127, so their K=64 score matmuls auto-derive
    tile_position (0,0)/(64,0) and run CONCURRENTLY on disjoint PE row
    groups when issued back-to-back (halves score time on HW; the sim cost
    model doesn't model tile concurrency); (b) kv-proj of chunk c+1 is
    software-pipelined into the attention t-loop of chunk c, so the PE has
    fill work while ACT (exp) gates the attention pipeline; (c) per
    (pair, lc) the two heads' scores land in one [128, 1024] PSUM tile so
    a single exp activation covers both heads.

    PSUM budget: spt 2 tiles x 2 banks + apts 3 x 1 bank + kvproj 1 = 8."""
    NCH = nch
    NT_PER = NCH // P
    NCHUNKS = N // NCH
    nc = bacc.Bacc("TRN2", target_bir_lowering=False, debug=False, num_devices=8)
    xT = nc.dram_tensor("xT", [C, N], F32R, kind="ExternalInput").ap()
    queryT = nc.dram_tensor("queryT", [C, L], F32R, kind="ExternalInput").ap()
    wkT = nc.dram_tensor("wkT", [C, W], F32R, kind="ExternalInput").ap()
    wvT = nc.dram_tensor("wvT", [C, W], F32R, kind="ExternalInput").ap()
    wqT = nc.dram_tensor("wqT", [C, W], F32R, kind="ExternalInput").ap()
    wprojT = nc.dram_tensor("wprojT", [W, C], F32R, kind="ExternalInput").ap()
    y = nc.dram_tensor("y", [L, C], F32, kind="ExternalOutput").ap()

    xT3 = xT.rearrange("(ko ki) n -> ki ko n", ki=P)
    queryT3 = queryT.rearrange("(ko ki) l -> ki ko l", ki=P)
    wkT3 = wkT.rearrange("(ko ki) m -> ki ko m", ki=P)
    wvT3 = wvT.rearrange("(ko ki) m -> ki ko m", ki=P)
    wqT3 = wqT.rearrange("(ko ki) m -> ki ko m", ki=P)
    wprojT3 = wprojT.rearrange("(ko ki) c -> ki ko c", ki=P)

    with tile.TileContext(nc) as tc, ExitStack() as ctx:
      for rep in range(reps):
        persist_cm = tc.tile_pool(name=f"v4p{rep}", bufs=1)
        persist = persist_cm.__enter__()
        qT = persist.tile([P, 4, L], F32R, tag="qT")
        aoT = persist.tile([P, 4, L], F32R, tag="aoT")
        accumA = persist.tile([D + 1, 2 * HPC, 512], F32, tag="accumA")
        onesrc = persist.tile([P, 1], F32, tag="onesrc")
        nc.any.memset(onesrc[:], 1.0)
        nc.any.memset(accumA[:], 0.0)

        wp_cm = tc.tile_pool(name=f"v4w{rep}", bufs=1)
        wp = wp_cm.__enter__()
        wkr = wp.tile([P, 8, W], F32R, tag="wk")
        nc.sync.dma_start(wkr[:], wkT3[:])
        wvr = wp.tile([P, 8, W], F32R, tag="wv")
        nc.sync.dma_start(wvr[:], wvT3[:])

        xstg_cm = tc.tile_pool(name=f"v4x{rep}", bufs=2)
        xstg = xstg_cm.__enter__()
        kvp_cm = tc.tile_pool(name=f"v4kv{rep}", bufs=2)
        kvp = kvp_cm.__enter__()

        def load_x(c):
            xr = xstg.tile([P, 8, NCH], F32R, tag="xs", name=f"xs_{rep}_{c}")
            nc.sync.dma_start(xr[:], xT3[:, :, c * NCH:(c + 1) * NCH])
            return xr

        def new_kv_tiles(c):
            kTc = kvp.tile([P, 4, NCH], F32R, tag="kTc", name=f"kTc_{rep}_{c}")
            v520c = kvp.tile([P, NT_PER, HPC, D + 1], F32R, tag="v520c",
                             name=f"v520c_{rep}_{c}")
            nc.vector.tensor_copy(
                v520c[:, :, :, D],
                onesrc[:, 0:1].to_broadcast([P, NT_PER, HPC]))
            return kTc, v520c

        def kv_group(pp, xr, kTc, v520c, c, g):
            """Emit kv-proj psum group g (0-3: kproj m, 4-7: vproj t)."""
            if g < 4:
                m = g
                ps = pp.tile([P, NCH], F32, tag="kvpsum", name=f"kps_{rep}_{c}_{m}")
                for k in range(8):
                    nc.tensor.matmul(ps[:], wkr[:, k, m * P:(m + 1) * P],
                                     xr[:, k, :], start=(k == 0), stop=(k == 7))
                nc.vector.tensor_copy(kTc[:, m, :], ps[:])
            else:
                t = g - 4
                ps = pp.tile([P, W], F32, tag="kvpsum", name=f"vps_{rep}_{c}_{t}")
                for k in range(8):
                    nc.tensor.matmul(ps[:], xr[:, k, t * P:(t + 1) * P],
                                     wvr[:, k, :], start=(k == 0), stop=(k == 7))
                nc.vector.tensor_copy(
                    v520c[:, t, :, 0:D],
                    ps[:].rearrange("p (h d) -> p h d", h=HPC))

        # ---- prologue: qproj + kvproj(chunk 0), interleaved ----
        xr_cur = load_x(0)
        kv_cur = new_kv_tiles(0)
        with tc.tile_pool(name=f"v4qw{rep}", bufs=1) as qwp, \
             tc.tile_pool(name=f"v4qs{rep}", bufs=1) as qstg, \
             tc.tile_pool(name=f"v4pp{rep}", bufs=4, space="PSUM") as ppp:
            wqr = qwp.tile([P, 8, W], F32R, tag="wq")
            nc.sync.dma_start(wqr[:], wqT3[:])
            qr = None
            for g in range(8):
                lc, m = g // 4, g % 4
                if m == 0:
                    qr = qstg.tile([P, 8, 512], F32R, tag="qs",
                                   name=f"qs_{rep}_{lc}")
                    nc.sync.dma_start(qr[:],
                                      queryT3[:, :, lc * 512:(lc + 1) * 512])
                kv_group(ppp, xr_cur, kv_cur[0], kv_cur[1], 0, g)
                ps = ppp.tile([P, 512], F32, tag="qps", name=f"qps_{rep}_{g}")
                for k in range(8):
                    nc.tensor.matmul(ps[:], wqr[:, k, m * P:(m + 1) * P],
                                     qr[:, k, :], start=(k == 0), stop=(k == 7))
                nc.vector.tensor_copy(qT[:, m, lc * 512:(lc + 1) * 512], ps[:])

        # ---- main loop: attention(c) interleaved with kvproj(c+1) ----
        with tc.tile_pool(name=f"v4pr{rep}", bufs=3) as probs_pool, \
             tc.tile_pool(name=f"v4sm{rep}", bufs=2) as small, \
             tc.tile_pool(name=f"v4kvps{rep}", bufs=1, space="PSUM") as kvps, \
             tc.tile_pool(name=f"v4sps{rep}", bufs=2, space="PSUM") as spsum, \
             tc.tile_pool(name=f"v4aps{rep}", bufs=3, space="PSUM") as apsum:
            for c in range(NCHUNKS):
                xr_att, (kTc, v520c) = xr_cur, kv_cur
                if c + 1 < NCHUNKS:
                    xr_cur = load_x(c + 1)
                    kv_cur = new_kv_tiles(c + 1)
                for pr in range(4):
                    for lc in range(2):
                        g = pr * 2 + lc
                        if c + 1 < NCHUNKS:
                            kv_group(kvps, xr_cur, kv_cur[0], kv_cur[1], c + 1, g)
                        apA = apsum.tile([D + 1, 512], F32, tag="apt",
                                         name=f"apA_{rep}_{c}_{g}")
                        apB = apsum.tile([D + 1, 512], F32, tag="apt",
                                         name=f"apB_{rep}_{c}_{g}")
                        for t in range(NT_PER):
                            spt = spsum.tile([P, 1024], F32, tag="spt")
                            # the two heads' score matmuls are issued
                            # back-to-back on disjoint PE row groups ->
                            # concurrent on HW (tile_position (0,0)/(64,0))
                            nc.tensor.matmul(
                                spt[:, 0:512],
                                kTc[0:D, pr, t * P:(t + 1) * P],
                                qT[0:D, pr, lc * 512:(lc + 1) * 512],
                                start=True, stop=True)
                            nc.tensor.matmul(
                                spt[:, 512:1024],
                                kTc[D:P, pr, t * P:(t + 1) * P],
                                qT[D:P, pr, lc * 512:(lc + 1) * 512],
                                start=True, stop=True)
                            pt = probs_pool.tile([P, 1024], F32R, tag="pt")
                            nc.scalar.activation(pt[:], spt[:], EXP, scale=SCALE)
                            nc.tensor.matmul(
                                apA[:], v520c[:, t, 2 * pr, :],
                                pt[:, 0:512],
                                start=(t == 0), stop=(t == NT_PER - 1))
                            nc.tensor.matmul(
                                apB[:], v520c[:, t, 2 * pr + 1, :],
                                pt[:, 512:1024],
                                start=(t == 0), stop=(t == NT_PER - 1))
                        idxA = (2 * pr) * 2 + lc
                        idxB = (2 * pr + 1) * 2 + lc
                        nc.vector.tensor_add(accumA[:, idxA, :],
                                             accumA[:, idxA, :], apA[:])
                        nc.vector.tensor_add(accumA[:, idxB, :],
                                             accumA[:, idxB, :], apB[:])
            # normalize + write aoT
            for h in range(HPC):
                bp = D * (h % 2)
                pr = h // 2
                for lc in range(2):
                    idx = h * 2 + lc
                    r64t = small.tile([P, 512], F32, tag="r64")
                    nc.vector.reciprocal(r64t[D:D + 1, :], accumA[D:D + 1, idx, :])
                    rrow = small.tile([1, 512], F32, tag="rrow")
                    nc.sync.dma_start(rrow[:], r64t[D:D + 1, :])
                    rb = small.tile([D, 512], F32, tag="rb")
                    nc.gpsimd.partition_broadcast(rb[:], rrow[:])
                    dst = aoT[bp:bp + D, pr, lc * 512:(lc + 1) * 512]
                    if bp == 0:
                        nc.vector.tensor_mul(dst, accumA[0:D, idx, :], rb[:])
                    else:
                        tmp = small.tile([D, 512], F32R, tag="aotmp")
                        nc.vector.tensor_mul(tmp[:], accumA[0:D, idx, :], rb[:])
                        nc.sync.dma_start(dst, tmp[:])

        kvp_cm.__exit__(None, None, None)
        xstg_cm.__exit__(None, None, None)
        wp_cm.__exit__(None, None, None)

        # ---- proj ----
        with tc.tile_pool(name=f"v4pw{rep}", bufs=1) as pwp, \
             tc.tile_pool(name=f"v4y{rep}", bufs=3) as ypool, \
             tc.tile_pool(name=f"v4yp{rep}", bufs=3, space="PSUM") as pp:
            wpr = pwp.tile([P, 4, C], F32R, tag="wp")
            nc.sync.dma_start(wpr[:], wprojT3[:])
            for l in range(L // P):
                for co in range(2):
                    ps = pp.tile([P, 512], F32, tag="yps")
                    for ci in range(4):
                        nc.tensor.matmul(ps[:], aoT[:, ci, l * P:(l + 1) * P],
                                         wpr[:, ci, co * 512:(co + 1) * 512],
                                         start=(ci == 0), stop=(ci == 3))
                    yt = ypool.tile([P, 512], F32, tag="yt")
                    nc.vector.tensor_copy(yt[:], ps[:])
                    nc.sync.dma_start(y[l * P:(l + 1) * P, co * 512:(co + 1) * 512],
                                      yt[:])
        persist_cm.__exit__(None, None, None)
    nc.finalize()
    return nc


BUILD = build_v4

_NC_CACHE = {}


def get_nc():
    if "nc" not in _NC_CACHE:
        _NC_CACHE["nc"] = BUILD()
    return _NC_CACHE["nc"]


def make_in_maps(x, query, Wq, Wkv, Wproj):
    x = np.asarray(x, dtype=np.float32)
    query = np.asarray(query, dtype=np.float32)
    Wq = np.asarray(Wq, dtype=np.float32)
    Wkv = np.asarray(Wkv, dtype=np.float32)
    Wproj = np.asarray(Wproj, dtype=np.float32)
    in_maps = []
    for core in range(8):
        b, half = core // 2, core % 2
        h0 = half * W  # 0 or 512: channel offset of this core's heads
        in_maps.append({
            "xT": np.ascontiguousarray(x[b].T),
            "queryT": np.ascontiguousarray(query[b].T),
            "wkT": np.ascontiguousarray(Wkv[h0:h0 + W, :].T),
            "wvT": np.ascontiguousarray(Wkv[C + h0:C + h0 + W, :].T),
            "wqT": np.ascontiguousarray(Wq[h0:h0 + W, :].T),
            "wprojT": np.ascontiguousarray(Wproj[:, h0:h0 + W].T),
        })
    return in_maps


def combine(results, bproj):
    y = np.zeros((B, L, C), np.float32)
    for core in range(8):
        y[core // 2] += results[core]["y"]
    y += np.asarray(bproj, dtype=np.float32)[None, None, :]
    return y


def kernel(x, query, Wq, Wkv, Wproj, bproj):
    nc = get_nc()
    in_maps = make_in_maps(x, query, Wq, Wkv, Wproj)
    res = run_bass_kernel_spmd(nc, in_maps, core_ids=list(range(8)))
    return combine(res.results, bproj)

